# revision 32
# baseline (speedup 1.0000x reference)
"""Sparse attention (per-query top-K) Trainium2 kernel, 8-core tensor-parallel.

Strategy (heads sharded 2-per-core, dense-score formulation):
  - Host folds idx/valid/geo_bias into per-(s,q) merged bias factors
    E[s,q] = sum_{j: idx[q,j]==s} exp(geo_bias[h,q,j]), stored as causal
    fp16 tiles.  This turns the per-query gather/softmax into dense math:
        A^T = E^T * exp(S^T - C),   S^T = K @ Q^T (feature-major)
        out^T = [V | 1]^T @ A^T     (row 64 = softmax denominator)
  - All matmul MOVING operands are fp16 (measured ~0.85 ns/row vs ~1.07
    for fp32r / ~1.14 for bf16 on this part).  x arrives pre-transposed
    fp16 from the host (no on-device DMA transpose).
  - Diagonal chunks only compute scores/exp for the causally live query
    range (E is zero elsewhere; the skipped z region is memset to 0 so
    the full-width E-multiply stays NaN-free).
  - Head outputs are normalized per query tile (denominator row 64 of
    the AV PSUM -> reciprocal -> PE broadcast -> DVE multiply) before
    the all-to-all, so the reshard carries [128, 512] fp16 per tile and
    phase 3 is o_proj only.
"""

import sys

sys.path.insert(0, "/opt/trn_rl_repo")

import numpy as np
import ml_dtypes

from concourse import bacc, mybir, tile
from concourse.bass_utils import run_bass_kernel_spmd
from concourse.masks import make_identity

F32 = mybir.dt.float32
F16 = mybir.dt.float16

S = 4096
H = 1024
NH = 16
KSEL = 32
HD = 64
NC = 8
HPC = NH // NC  # 2 heads per core
QT = 512
NQT = S // QT
SC = 128
CSHIFT = 2.0
SLAB = 8  # s-chunks per E-tile DMA slab

TILE_LIST = [(t, c) for t in range(NQT) for c in range(4 * (t + 1))]
N_TILES = len(TILE_LIST)  # 144
TILE_IDX = {tc: n for n, tc in enumerate(TILE_LIST)}


def _build_program(n_cores_build=NC):
    nc = bacc.Bacc(
        "TRN2", target_bir_lowering=False, debug=False, num_devices=n_cores_build
    )

    xT_in = nc.dram_tensor("xT", [128, 8, S], F16, kind="ExternalInput").ap()
    wq_in = nc.dram_tensor("wq", [128, 8, 128], F16, kind="ExternalInput").ap()
    wk_in = nc.dram_tensor("wk", [128, 8, 128], F16, kind="ExternalInput").ap()
    wv_in = nc.dram_tensor("wv", [128, 8, 128], F16, kind="ExternalInput").ap()
    wo_in = nc.dram_tensor("wo", [128, 8, H], F16, kind="ExternalInput").ap()
    bo_in = nc.dram_tensor("bo_rep", [128, H], F32, kind="ExternalInput").ap()
    sel16_in = nc.dram_tensor("sel16", [NH, H], F16, kind="ExternalInput").ap()
    # osel[:, 0] = 1 if this core uses the first (tiles 0-3) all-to-all,
    # osel[:, 1] = 1 - osel[:, 0]; per-core constant input
    osel_in = nc.dram_tensor("osel", [128, 2], F32, kind="ExternalInput").ap()
    e_in = nc.dram_tensor(
        "e_pack", [N_TILES, SC, HPC, QT], F16, kind="ExternalInput"
    ).ap()
    y_out = nc.dram_tensor("y_part", [QT, H], F32, kind="ExternalOutput").ap()

    with tile.TileContext(nc) as tc:
        with (
            tc.tile_pool(name="const", bufs=1) as constp,
            tc.tile_pool(name="persist", bufs=1) as persist,
            tc.tile_pool(name="dram", bufs=1, space="DRAM") as dram,
        ):
            ident = constp.tile([128, 128], F16, tag="ident")
            make_identity(nc, ident[:])
            nbias = constp.tile([128, 1], F32, tag="nbias")
            nc.gpsimd.memset(nbias[:], -CSHIFT)

            wq_sb = constp.tile([128, 8, 128], F16, tag="wq")
            wk_sb = constp.tile([128, 8, 128], F16, tag="wk")
            wv_sb = constp.tile([128, 8, 128], F16, tag="wv")
            nc.scalar.dma_start(wq_sb[:], wq_in[:])
            nc.scalar.dma_start(wk_sb[:], wk_in[:])
            nc.scalar.dma_start(wv_sb[:], wv_in[:])
            # wo/bo/sel16 are phase-3-only: their DMAs are emitted after
            # phase 1 so they don't delay the xT stream at startup
            wo_sb = constp.tile([128, 8, H], F16, tag="wo")
            bo_sb = constp.tile([128, H], F32, tag="bo")
            sel16_sb = constp.tile([NH, H], F16, tag="sel16")
            osel_sb = constp.tile([128, 2], F32, tag="osel")

            qT_sb = persist.tile([128, NQT, QT], F16, tag="qT")
            kT_sb = persist.tile([128, NQT, QT], F16, tag="kT")
            v_sb = [
                persist.tile([128, S // SC, HD + 1], F16, tag=f"v{h}", name=f"v{h}")
                for h in range(HPC)
            ]
            for h in range(HPC):
                nc.gpsimd.memset(v_sb[h][:], 1.0)

            a2a_in = [
                dram.tile([NC, HPC * (HD + 1), QT], F16, name=f"a2a_in{i}")
                for i in range(2)
            ]
            a2a_out = [
                dram.tile([NC, HPC * (HD + 1), QT], F16, name=f"a2a_out{i}")
                for i in range(2)
            ]

            # ------------- phase 1: projections (feature-major) ---------
            with (
                tc.tile_pool(name="xT", bufs=3) as xTp,
                tc.tile_pool(name="vt", bufs=2) as vtmpp,
                tc.tile_pool(name="p1ps", bufs=2, space="PSUM") as p1ps,
            ):
                for st in range(NQT):
                    sl = slice(st * QT, (st + 1) * QT)
                    xh = xTp.tile([128, 8, QT], F16, tag="xh")
                    nc.sync.dma_start(xh[:], xT_in[:, :, sl])
                    ps_q = p1ps.tile([128, QT], F32, tag="psq")
                    ps_k = p1ps.tile([128, QT], F32, tag="psk")
                    ps_v = p1ps.tile([128, QT], F32, tag="psv")
                    for c in range(8):
                        nc.tensor.matmul(
                            ps_q[:], wq_sb[:, c, :], xh[:, c, :],
                            start=(c == 0), stop=(c == 7),
                        )
                        nc.tensor.matmul(
                            ps_k[:], wk_sb[:, c, :], xh[:, c, :],
                            start=(c == 0), stop=(c == 7),
                        )
                        nc.tensor.matmul(
                            ps_v[:], wv_sb[:, c, :], xh[:, c, :],
                            start=(c == 0), stop=(c == 7),
                        )
                    nc.vector.tensor_copy(qT_sb[:, st, :], ps_q[:])
                    nc.vector.tensor_copy(kT_sb[:, st, :], ps_k[:])
                    vT_tmp = vtmpp.tile([128, QT], F16, tag="vt")
                    nc.scalar.copy(vT_tmp[:], ps_v[:])
                    ps_tv = p1ps.tile([128, QT], F16, tag="tp")
                    for i in range(4):
                        nc.tensor.transpose(
                            ps_tv[:, i * 128 : (i + 1) * 128],
                            vT_tmp[:, i * 128 : (i + 1) * 128],
                            ident[:],
                        )
                    ps_tv4 = ps_tv[:].rearrange("p (i h d) -> p i h d", i=4, h=HPC)
                    for h in range(HPC):
                        nc.vector.tensor_copy(
                            v_sb[h][:, st * 4 : (st + 1) * 4, 0:HD],
                            ps_tv4[:, :, h, :],
                        )

            # phase-3 weight loads, deferred past the xT stream
            nc.scalar.dma_start(wo_sb[:], wo_in[:])
            nc.scalar.dma_start(bo_sb[:], bo_in[:])
            nc.scalar.dma_start(sel16_sb[:], sel16_in[:])
            nc.scalar.dma_start(osel_sb[:], osel_in[:])

            # ------------- phase 2: attention ---------------------------
            # PE stream is software-pipelined: the AV matmuls of chunk c are
            # emitted after the score matmuls of chunk c+2, so the in-order
            # PE queue never stalls on the exp->mult dependency chain.
            LAG = 2
            p3pool = tc.tile_pool(name="p3", bufs=1)
            p3 = p3pool.__enter__()
            # den row order: l*8 + ci  (head h = 2*ci + l).  Each core blends
            # the two half-collectives' outputs with its osel mask (the
            # unused half holds duplicated finite data, so 0-masking is
            # NaN-safe).
            den_h = [
                p3.tile([NH, QT], F16, tag=f"den{i}", name=f"den{i}")
                for i in range(2)
            ]
            oT_h = [
                p3.tile([128, 8, QT], F16, tag=f"oT{i}", name=f"oT{i}")
                for i in range(2)
            ]
            with (
                tc.tile_pool(name="zap", bufs=6) as zap,
                tc.tile_pool(name="ep", bufs=3) as epool,
                tc.tile_pool(name="otp", bufs=2) as otp,
                tc.tile_pool(name="p2s", bufs=3, space="PSUM") as p2s,
                tc.tile_pool(name="p2o", bufs=1, space="PSUM") as p2o,
            ):
                slab_no = 0
                for t in reversed(range(NQT)):
                    nchunks = 4 * (t + 1)
                    slabs = []
                    for g0 in range(0, nchunks, SLAB):
                        gsz = min(SLAB, nchunks - g0)
                        e_slab = epool.tile(
                            [128, SLAB, HPC, QT], F16, tag="e", name="e_slab"
                        )
                        n0 = TILE_IDX[(t, g0)]
                        src = e_in[n0 : n0 + gsz].rearrange("n p h q -> p n h q")
                        # e-slab DMAs stay off gpsimd so the early collective
                        # cannot block them
                        if slab_no % 2 == 0:
                            nc.sync.dma_start(e_slab[:, 0:gsz, :, :], src)
                        else:
                            nc.scalar.dma_start(e_slab[:, 0:gsz, :, :], src)
                        slab_no += 1
                        slabs.append(e_slab)
                    ps_o = [
                        p2o.tile([HD + 1, QT], F32, tag=f"po{h}", name=f"po{h}")
                        for h in range(HPC)
                    ]
                    a_tiles = {}

                    def emit_av(c):
                        a_sb = a_tiles.pop(c)
                        for h in range(HPC):
                            nc.tensor.matmul(
                                ps_o[h][:],
                                v_sb[h][:, c, :],
                                a_sb[:, h, :],
                                start=(c == 0),
                                stop=(c == nchunks - 1),
                            )

                    for c in range(nchunks):
                        e_slab = slabs[c // SLAB]
                        c_loc = c % SLAB
                        # causal trim: in the 4 diagonal chunks only queries
                        # >= qlo can select keys of this chunk
                        qlo = max(0, (c - 4 * t)) * 128
                        ps_s2 = p2s.tile([128, HPC, QT], F32, tag="ps2")
                        for h in range(HPC):
                            nc.tensor.matmul(
                                ps_s2[:, h, qlo:],
                                kT_sb[
                                    h * HD : (h + 1) * HD,
                                    c // 4,
                                    (c % 4) * 128 : (c % 4 + 1) * 128,
                                ],
                                qT_sb[h * HD : (h + 1) * HD, t, qlo:],
                                start=True,
                                stop=True,
                            )
                        z_sb = zap.tile([128, HPC, QT], F16, tag="z")
                        if qlo > 0:
                            nc.vector.memset(z_sb[:, :, 0:qlo], 0.0)
                        nc.scalar.activation(
                            z_sb[:, :, qlo:],
                            ps_s2[:, :, qlo:],
                            mybir.ActivationFunctionType.Exp,
                            bias=nbias[:],
                        )
                        a_sb = zap.tile([128, HPC, QT], F16, tag="a")
                        nc.vector.tensor_mul(a_sb[:], z_sb[:], e_slab[:, c_loc, :, :])
                        a_tiles[c] = a_sb
                        if c >= LAG:
                            emit_av(c - LAG)
                    for c in range(max(0, nchunks - LAG), nchunks):
                        emit_av(c)
                    ot_sb = otp.tile([HD + 1, HPC, QT], F16, tag="ot")
                    for h in range(HPC):
                        nc.vector.tensor_copy(ot_sb[:, h, :], ps_o[h][:])
                    # each half-collective's unused slots get a duplicate of a
                    # real tile so no receiver reads uninitialized DRAM
                    buf = a2a_in[t // 4]
                    for slot in (t, (t + 4) % NQT):
                        nc.sync.dma_start(
                            buf[slot].rearrange("(h p) q -> p h q", h=HPC), ot_sb[:]
                        )
                    if t == 4:
                        # first half-collective (tiles 4-7) overlaps the
                        # remaining tiles 3..0 (gpsimd queue is otherwise
                        # idle here); its unpack DMAs overlap too
                        nc.gpsimd.collective_compute(
                            "AllToAll",
                            mybir.AluOpType.bypass,
                            replica_groups=[list(range(NC))],
                            ins=[a2a_in[1].opt()],
                            outs=[a2a_out[1].opt()],
                        )
                        for l in range(HPC):
                            nc.sync.dma_start(
                                den_h[1][l * 8 : (l + 1) * 8, :],
                                a2a_out[1][:, l * (HD + 1) + HD, :],
                            )
                            nc.sync.dma_start(
                                oT_h[1][l * HD : (l + 1) * HD, :, :],
                                a2a_out[1][
                                    :, l * (HD + 1) : l * (HD + 1) + HD, :
                                ].rearrange("c d q -> d c q"),
                            )

            nc.gpsimd.collective_compute(
                "AllToAll",
                mybir.AluOpType.bypass,
                replica_groups=[list(range(NC))],
                ins=[a2a_in[0].opt()],
                outs=[a2a_out[0].opt()],
            )

            # ------------- phase 3: normalize + o_proj ------------------
            with (
                tc.tile_pool(name="p3y", bufs=2) as p3y,
                tc.tile_pool(name="p3ps", bufs=2, space="PSUM") as p3ps,
            ):
                for l in range(HPC):
                    nc.sync.dma_start(
                        den_h[0][l * 8 : (l + 1) * 8, :],
                        a2a_out[0][:, l * (HD + 1) + HD, :],
                    )
                    nc.sync.dma_start(
                        oT_h[0][l * HD : (l + 1) * HD, :, :],
                        a2a_out[0][
                            :, l * (HD + 1) : l * (HD + 1) + HD, :
                        ].rearrange("c d q -> d c q"),
                    )
                oT_sb = p3.tile([128, 8, QT], F16, tag="oT")
                nc.vector.tensor_scalar_mul(oT_sb[:], oT_h[1][:], osel_sb[:, 1:2])
                nc.vector.scalar_tensor_tensor(
                    oT_sb[:],
                    oT_h[0][:],
                    osel_sb[:, 0:1],
                    oT_sb[:],
                    op0=mybir.AluOpType.mult,
                    op1=mybir.AluOpType.add,
                )
                den_sb = p3.tile([NH, QT], F16, tag="den")
                nc.vector.tensor_scalar_mul(
                    den_sb[:], den_h[1][:], osel_sb[0:NH, 1:2]
                )
                nc.vector.scalar_tensor_tensor(
                    den_sb[:],
                    den_h[0][:],
                    osel_sb[0:NH, 0:1],
                    den_sb[:],
                    op0=mybir.AluOpType.mult,
                    op1=mybir.AluOpType.add,
                )
                rden_sb = p3.tile([NH, QT], F16, tag="rden")
                with nc.allow_low_precision(reason="fp16 reciprocal broadcast"):
                    nc.vector.reciprocal(rden_sb[:], den_sb[:])

                on_sb = p3.tile([128, 8, QT], F16, tag="on")
                for ci in range(8):
                    ps_b = p3ps.tile([128, QT], F32, tag="bc")
                    nc.tensor.matmul(
                        ps_b[:],
                        sel16_sb[:, ci * 128 : (ci + 1) * 128],
                        rden_sb[:],
                        start=True,
                        stop=True,
                    )
                    nc.vector.tensor_mul(on_sb[:, ci, :], oT_sb[:, ci, :], ps_b[:])

                for qb in range(4):
                    y_sb = p3y.tile([128, H], F32, tag="y")
                    for fh in range(2):
                        ps_y = p3ps.tile([128, QT], F32, tag="py")
                        for c in range(8):
                            nc.tensor.matmul(
                                ps_y[:],
                                on_sb[:, c, qb * 128 : (qb + 1) * 128],
                                wo_sb[:, c, fh * QT : (fh + 1) * QT],
                                start=(c == 0),
                                stop=(c == 7),
                            )
                        nc.vector.tensor_add(
                            y_sb[:, fh * QT : (fh + 1) * QT],
                            ps_y[:],
                            bo_sb[:, fh * QT : (fh + 1) * QT],
                        )
                    nc.sync.dma_start(y_out[qb * 128 : (qb + 1) * 128, :], y_sb[:])
            p3pool.__exit__(None, None, None)

    nc.compile()
    return nc


_PROGRAM_CACHE = {}


def _get_program():
    if "nc" not in _PROGRAM_CACHE:
        _PROGRAM_CACHE["nc"] = _build_program()
    return _PROGRAM_CACHE["nc"]


def _host_prep(x, idx, valid, geo_bias, Wq, Wk, Wv, Wo, bo):
    x2 = np.ascontiguousarray(np.asarray(x, dtype=np.float32).reshape(S, H))
    idx = np.asarray(idx).astype(np.int64)
    valid = np.asarray(valid).astype(bool)
    geo = np.asarray(geo_bias, dtype=np.float32)
    Wq = np.asarray(Wq, dtype=np.float32)
    Wk = np.asarray(Wk, dtype=np.float32)
    Wv = np.asarray(Wv, dtype=np.float32)
    Wo = np.asarray(Wo, dtype=np.float32)
    bo = np.asarray(bo, dtype=np.float32)

    qpos = np.arange(S, dtype=np.int64)[:, None]
    keep = valid & (idx <= qpos) & (idx >= 0)
    s_flat = idx[keep]
    q_flat = np.broadcast_to(qpos, idx.shape)[keep]
    lin = s_flat * S + q_flat

    bo_rep = np.ascontiguousarray(np.broadcast_to(bo[None, :], (128, H)))

    # den row order in phase 3 is r = l*8 + ci for head h = 2*ci + l
    sel16 = np.zeros((NH, H), dtype=np.float16)
    ch = np.arange(H)
    sel16[((ch // HD) % 2) * 8 + ch // 128, ch] = 1.0

    wq_scaled = Wq / np.sqrt(HD)
    # xT[p, c, s] = x[s, c*128+p]
    xT = np.ascontiguousarray(
        x2.T.reshape(8, 128, S).transpose(1, 0, 2)
    ).astype(np.float16)

    def wpack(W, cs):
        # w[p, c, m] = W[c*128+p, cols[m]]
        return np.ascontiguousarray(
            W[:, cs].reshape(8, 128, -1).transpose(1, 0, 2)
        ).astype(np.float16)

    wo_pack = np.ascontiguousarray(
        Wo.reshape(8, 128, H).transpose(1, 0, 2)
    ).astype(np.float16)

    in_maps = []
    for core in range(NC):
        e_pack = np.empty((N_TILES, SC, HPC, QT), dtype=np.float16)
        for l in range(HPC):
            h = HPC * core + l
            w = np.exp(geo[h][keep].astype(np.float64))
            eT = np.bincount(lin, weights=w, minlength=S * S).reshape(S, S)
            for n, (t, c) in enumerate(TILE_LIST):
                e_pack[n, :, l, :] = eT[
                    c * SC : (c + 1) * SC, t * QT : (t + 1) * QT
                ].astype(np.float16)
        cs = slice(128 * core, 128 * (core + 1))
        osel = np.zeros((128, 2), dtype=np.float32)
        osel[:, 0 if core < 4 else 1] = 1.0
        in_maps.append(
            {
                "xT": xT,
                "wq": wpack(wq_scaled, cs),
                "wk": wpack(Wk, cs),
                "wv": wpack(Wv, cs),
                "wo": wo_pack,
                "bo_rep": bo_rep,
                "sel16": sel16,
                "osel": osel,
                "e_pack": e_pack,
            }
        )
    return in_maps


def kernel(x, idx, valid, geo_bias, Wq, Wk, Wv, Wo, bo):
    b, s, h = np.asarray(x).shape
    assert (b, s, h) == (1, S, H)
    in_maps = _host_prep(x, idx, valid, geo_bias, Wq, Wk, Wv, Wo, bo)
    nc = _get_program()
    res = run_bass_kernel_spmd(nc, in_maps, core_ids=list(range(NC)))
    y = np.concatenate([res.results[c]["y_part"] for c in range(NC)], axis=0)
    return y.reshape(1, S, H).astype(np.float32)


# revision 33
# speedup vs baseline: 1.1779x; 1.1779x over previous
"""Sparse attention (per-query top-K) Trainium2 kernel, 8-core tensor-parallel.

Strategy (heads sharded 2-per-core, dense-score formulation):
  - Host folds idx/valid/geo_bias into per-(s,q) merged bias factors
    E[s,q] = sum_{j: idx[q,j]==s} exp(geo_bias[h,q,j]), stored as causal
    fp16 tiles.  This turns the per-query gather/softmax into dense math:
        A^T = E^T * exp(S^T - C),   S^T = K @ Q^T (feature-major)
        out^T = [V | 1]^T @ A^T     (row 64 = softmax denominator)
  - All matmul MOVING operands are fp16 (measured ~0.85 ns/row vs ~1.07
    for fp32r / ~1.14 for bf16 on this part).  x arrives pre-transposed
    fp16 from the host (no on-device DMA transpose).
  - Diagonal chunks only compute scores/exp for the causally live query
    range (E is zero elsewhere; the skipped z region is memset to 0 so
    the full-width E-multiply stays NaN-free).
  - Head outputs are normalized per query tile (denominator row 64 of
    the AV PSUM -> reciprocal -> PE broadcast -> DVE multiply) before
    the all-to-all, so the reshard carries [128, 512] fp16 per tile and
    phase 3 is o_proj only.
"""

import sys

sys.path.insert(0, "/opt/trn_rl_repo")

import numpy as np
import ml_dtypes

from concourse import bacc, mybir, tile
from concourse.bass_utils import run_bass_kernel_spmd
from concourse.masks import make_identity

F32 = mybir.dt.float32
F16 = mybir.dt.float16

S = 4096
H = 1024
NH = 16
KSEL = 32
HD = 64
NC = 8
HPC = NH // NC  # 2 heads per core
QT = 512
NQT = S // QT
SC = 128
CSHIFT = 2.0
SLAB = 8  # s-chunks per E-tile DMA slab

TILE_LIST = [(t, c) for t in range(NQT) for c in range(4 * (t + 1))]
N_TILES = len(TILE_LIST)  # 144
TILE_IDX = {tc: n for n, tc in enumerate(TILE_LIST)}


def _build_program(n_cores_build=NC):
    nc = bacc.Bacc(
        "TRN2", target_bir_lowering=False, debug=False, num_devices=n_cores_build
    )

    xT_in = nc.dram_tensor("xT", [128, 8, S], F16, kind="ExternalInput").ap()
    wq_in = nc.dram_tensor("wq", [128, 8, 128], F16, kind="ExternalInput").ap()
    wk_in = nc.dram_tensor("wk", [128, 8, 128], F16, kind="ExternalInput").ap()
    wv_in = nc.dram_tensor("wv", [128, 8, 128], F16, kind="ExternalInput").ap()
    wo_in = nc.dram_tensor("wo", [128, 8, H], F16, kind="ExternalInput").ap()
    bo_in = nc.dram_tensor("bo_rep", [128, H], F32, kind="ExternalInput").ap()
    sel16_in = nc.dram_tensor("sel16", [NH, H], F16, kind="ExternalInput").ap()
    # osel[:, 0] = 1 if this core uses the first (tiles 0-3) all-to-all,
    # osel[:, 1] = 1 - osel[:, 0]; per-core constant input
    osel_in = nc.dram_tensor("osel", [128, 2], F32, kind="ExternalInput").ap()
    e_in = nc.dram_tensor(
        "e_pack", [N_TILES, SC, HPC, QT], F16, kind="ExternalInput"
    ).ap()
    y_out = nc.dram_tensor("y_part", [QT, H], F32, kind="ExternalOutput").ap()

    with tile.TileContext(nc) as tc:
        with (
            tc.tile_pool(name="const", bufs=1) as constp,
            tc.tile_pool(name="persist", bufs=1) as persist,
            tc.tile_pool(name="dram", bufs=1, space="DRAM") as dram,
        ):
            ident = constp.tile([128, 128], F16, tag="ident")
            make_identity(nc, ident[:])
            nbias = constp.tile([128, 1], F32, tag="nbias")
            nc.gpsimd.memset(nbias[:], -CSHIFT)

            wq_sb = constp.tile([128, 8, 128], F16, tag="wq")
            wk_sb = constp.tile([128, 8, 128], F16, tag="wk")
            wv_sb = constp.tile([128, 8, 128], F16, tag="wv")
            nc.scalar.dma_start(wq_sb[:], wq_in[:])
            nc.scalar.dma_start(wk_sb[:], wk_in[:])
            nc.scalar.dma_start(wv_sb[:], wv_in[:])
            # wo/bo/sel16 are phase-3-only: their DMAs are emitted after
            # phase 1 so they don't delay the xT stream at startup
            wo_sb = constp.tile([128, 8, H], F16, tag="wo")
            bo_sb = constp.tile([128, H], F32, tag="bo")
            sel16_sb = constp.tile([NH, H], F16, tag="sel16")
            osel_sb = constp.tile([128, 2], F32, tag="osel")

            qT_sb = persist.tile([128, NQT, QT], F16, tag="qT")
            kT_sb = persist.tile([128, NQT, QT], F16, tag="kT")
            v_sb = [
                persist.tile([128, S // SC, HD + 1], F16, tag=f"v{h}", name=f"v{h}")
                for h in range(HPC)
            ]
            for h in range(HPC):
                nc.gpsimd.memset(v_sb[h][:], 1.0)

            a2a_in = [
                dram.tile([NC, HPC * (HD + 1), QT], F16, name=f"a2a_in{i}")
                for i in range(2)
            ]
            a2a_out = [
                dram.tile([NC, HPC * (HD + 1), QT], F16, name=f"a2a_out{i}")
                for i in range(2)
            ]

            # ------------- phase 1: projections (feature-major) ---------
            with (
                tc.tile_pool(name="xT", bufs=3) as xTp,
                tc.tile_pool(name="vt", bufs=2) as vtmpp,
                tc.tile_pool(name="p1ps", bufs=2, space="PSUM") as p1ps,
            ):
                for st in range(NQT):
                    sl = slice(st * QT, (st + 1) * QT)
                    xh = xTp.tile([128, 8, QT], F16, tag="xh")
                    nc.sync.dma_start(xh[:], xT_in[:, :, sl])
                    ps_q = p1ps.tile([128, QT], F32, tag="psq")
                    ps_k = p1ps.tile([128, QT], F32, tag="psk")
                    ps_v = p1ps.tile([128, QT], F32, tag="psv")
                    for c in range(8):
                        nc.tensor.matmul(
                            ps_q[:], wq_sb[:, c, :], xh[:, c, :],
                            start=(c == 0), stop=(c == 7),
                        )
                        nc.tensor.matmul(
                            ps_k[:], wk_sb[:, c, :], xh[:, c, :],
                            start=(c == 0), stop=(c == 7),
                        )
                        nc.tensor.matmul(
                            ps_v[:], wv_sb[:, c, :], xh[:, c, :],
                            start=(c == 0), stop=(c == 7),
                        )
                    nc.vector.tensor_copy(qT_sb[:, st, :], ps_q[:])
                    nc.vector.tensor_copy(kT_sb[:, st, :], ps_k[:])
                    vT_tmp = vtmpp.tile([128, QT], F16, tag="vt")
                    nc.scalar.copy(vT_tmp[:], ps_v[:])
                    ps_tv = p1ps.tile([128, QT], F16, tag="tp")
                    for i in range(4):
                        nc.tensor.transpose(
                            ps_tv[:, i * 128 : (i + 1) * 128],
                            vT_tmp[:, i * 128 : (i + 1) * 128],
                            ident[:],
                        )
                    ps_tv4 = ps_tv[:].rearrange("p (i h d) -> p i h d", i=4, h=HPC)
                    for h in range(HPC):
                        nc.vector.tensor_copy(
                            v_sb[h][:, st * 4 : (st + 1) * 4, 0:HD],
                            ps_tv4[:, :, h, :],
                        )

            # phase-3 weight loads, deferred past the xT stream
            nc.scalar.dma_start(wo_sb[:], wo_in[:])
            nc.scalar.dma_start(bo_sb[:], bo_in[:])
            nc.scalar.dma_start(sel16_sb[:], sel16_in[:])
            nc.scalar.dma_start(osel_sb[:], osel_in[:])

            # ------------- phase 2: attention ---------------------------
            # PE stream is software-pipelined: the AV matmuls of chunk c are
            # emitted after the score matmuls of chunk c+2, so the in-order
            # PE queue never stalls on the exp->mult dependency chain.
            LAG = 2
            p3pool = tc.tile_pool(name="p3", bufs=1)
            p3 = p3pool.__enter__()
            # den row order: l*8 + ci  (head h = 2*ci + l).  Each core blends
            # the two half-collectives' outputs with its osel mask (the
            # unused half holds duplicated finite data, so 0-masking is
            # NaN-safe).
            den_h = [
                p3.tile([NH, QT], F16, tag=f"den{i}", name=f"den{i}")
                for i in range(2)
            ]
            oT_h = [
                p3.tile([128, 8, QT], F16, tag=f"oT{i}", name=f"oT{i}")
                for i in range(2)
            ]
            with (
                tc.tile_pool(name="zap", bufs=6) as zap,
                tc.tile_pool(name="ep", bufs=3) as epool,
                tc.tile_pool(name="otp", bufs=2) as otp,
                tc.tile_pool(name="p2s", bufs=3, space="PSUM") as p2s,
                tc.tile_pool(name="p2o", bufs=1, space="PSUM") as p2o,
            ):
                slab_no = 0
                for t in reversed(range(NQT)):
                    nchunks = 4 * (t + 1)
                    slabs = []
                    for g0 in range(0, nchunks, SLAB):
                        gsz = min(SLAB, nchunks - g0)
                        e_slab = epool.tile(
                            [128, SLAB, HPC, QT], F16, tag="e", name="e_slab"
                        )
                        n0 = TILE_IDX[(t, g0)]
                        src = e_in[n0 : n0 + gsz].rearrange("n p h q -> p n h q")
                        # e-slab DMAs stay off gpsimd so the early collective
                        # cannot block them
                        if slab_no % 2 == 0:
                            nc.sync.dma_start(e_slab[:, 0:gsz, :, :], src)
                        else:
                            nc.scalar.dma_start(e_slab[:, 0:gsz, :, :], src)
                        slab_no += 1
                        slabs.append(e_slab)
                    ps_o = [
                        p2o.tile([HD + 1, QT], F32, tag=f"po{h}", name=f"po{h}")
                        for h in range(HPC)
                    ]
                    a_tiles = {}

                    def emit_av(c):
                        a_sb = a_tiles.pop(c)
                        for h in range(HPC):
                            nc.tensor.matmul(
                                ps_o[h][:],
                                v_sb[h][:, c, :],
                                a_sb[:, h, :],
                                start=(c == 0),
                                stop=(c == nchunks - 1),
                            )

                    for c in range(nchunks):
                        e_slab = slabs[c // SLAB]
                        c_loc = c % SLAB
                        # causal trim: in the 4 diagonal chunks only queries
                        # >= qlo can select keys of this chunk
                        qlo = max(0, (c - 4 * t)) * 128
                        ps_s2 = p2s.tile([128, HPC, QT], F32, tag="ps2")
                        for h in range(HPC):
                            nc.tensor.matmul(
                                ps_s2[:, h, qlo:],
                                kT_sb[
                                    h * HD : (h + 1) * HD,
                                    c // 4,
                                    (c % 4) * 128 : (c % 4 + 1) * 128,
                                ],
                                qT_sb[h * HD : (h + 1) * HD, t, qlo:],
                                start=True,
                                stop=True,
                            )
                        z_sb = zap.tile([128, HPC, QT], F16, tag="z")
                        if qlo > 0:
                            nc.vector.memset(z_sb[:, :, 0:qlo], 0.0)
                        nc.scalar.activation(
                            z_sb[:, :, qlo:],
                            ps_s2[:, :, qlo:],
                            mybir.ActivationFunctionType.Exp,
                            bias=nbias[:],
                        )
                        a_sb = zap.tile([128, HPC, QT], F16, tag="a")
                        nc.vector.tensor_mul(a_sb[:], z_sb[:], e_slab[:, c_loc, :, :])
                        a_tiles[c] = a_sb
                        if c >= LAG:
                            emit_av(c - LAG)
                    for c in range(max(0, nchunks - LAG), nchunks):
                        emit_av(c)
                    ot_sb = otp.tile([HD + 1, HPC, QT], F16, tag="ot")
                    for h in range(HPC):
                        nc.vector.tensor_copy(ot_sb[:, h, :], ps_o[h][:])
                    # each half-collective's unused slots get a duplicate of a
                    # real tile so no receiver reads uninitialized DRAM
                    buf = a2a_in[t // 4]
                    for slot in (t, (t + 4) % NQT):
                        nc.sync.dma_start(
                            buf[slot].rearrange("(h p) q -> p h q", h=HPC), ot_sb[:]
                        )
                    if t == 4:
                        # first half-collective (tiles 4-7) overlaps the
                        # remaining tiles 3..0 (gpsimd queue is otherwise
                        # idle here); its unpack DMAs overlap too
                        nc.gpsimd.collective_compute(
                            "AllToAll",
                            mybir.AluOpType.bypass,
                            replica_groups=[list(range(NC))],
                            ins=[a2a_in[1].opt()],
                            outs=[a2a_out[1].opt()],
                        )
                        # unpack on the gpsimd queue: it waits on cc #1
                        # there without blocking the sync-queue e-slab stream
                        for l in range(HPC):
                            nc.gpsimd.dma_start(
                                den_h[1][l * 8 : (l + 1) * 8, :],
                                a2a_out[1][:, l * (HD + 1) + HD, :],
                            )
                            nc.gpsimd.dma_start(
                                oT_h[1][l * HD : (l + 1) * HD, :, :],
                                a2a_out[1][
                                    :, l * (HD + 1) : l * (HD + 1) + HD, :
                                ].rearrange("c d q -> d c q"),
                            )

            nc.gpsimd.collective_compute(
                "AllToAll",
                mybir.AluOpType.bypass,
                replica_groups=[list(range(NC))],
                ins=[a2a_in[0].opt()],
                outs=[a2a_out[0].opt()],
            )

            # ------------- phase 3: normalize + o_proj ------------------
            with (
                tc.tile_pool(name="p3y", bufs=2) as p3y,
                tc.tile_pool(name="p3ps", bufs=2, space="PSUM") as p3ps,
            ):
                for l in range(HPC):
                    nc.sync.dma_start(
                        den_h[0][l * 8 : (l + 1) * 8, :],
                        a2a_out[0][:, l * (HD + 1) + HD, :],
                    )
                    nc.sync.dma_start(
                        oT_h[0][l * HD : (l + 1) * HD, :, :],
                        a2a_out[0][
                            :, l * (HD + 1) : l * (HD + 1) + HD, :
                        ].rearrange("c d q -> d c q"),
                    )
                oT_sb = p3.tile([128, 8, QT], F16, tag="oT")
                nc.vector.tensor_scalar_mul(oT_sb[:], oT_h[1][:], osel_sb[:, 1:2])
                nc.vector.scalar_tensor_tensor(
                    oT_sb[:],
                    oT_h[0][:],
                    osel_sb[:, 0:1],
                    oT_sb[:],
                    op0=mybir.AluOpType.mult,
                    op1=mybir.AluOpType.add,
                )
                den_sb = p3.tile([NH, QT], F16, tag="den")
                nc.vector.tensor_scalar_mul(
                    den_sb[:], den_h[1][:], osel_sb[0:NH, 1:2]
                )
                nc.vector.scalar_tensor_tensor(
                    den_sb[:],
                    den_h[0][:],
                    osel_sb[0:NH, 0:1],
                    den_sb[:],
                    op0=mybir.AluOpType.mult,
                    op1=mybir.AluOpType.add,
                )
                rden_sb = p3.tile([NH, QT], F16, tag="rden")
                with nc.allow_low_precision(reason="fp16 reciprocal broadcast"):
                    nc.vector.reciprocal(rden_sb[:], den_sb[:])

                on_sb = p3.tile([128, 8, QT], F16, tag="on")
                for ci in range(8):
                    ps_b = p3ps.tile([128, QT], F32, tag="bc")
                    nc.tensor.matmul(
                        ps_b[:],
                        sel16_sb[:, ci * 128 : (ci + 1) * 128],
                        rden_sb[:],
                        start=True,
                        stop=True,
                    )
                    nc.vector.tensor_mul(on_sb[:, ci, :], oT_sb[:, ci, :], ps_b[:])

                for qb in range(4):
                    y_sb = p3y.tile([128, H], F32, tag="y")
                    for fh in range(2):
                        ps_y = p3ps.tile([128, QT], F32, tag="py")
                        for c in range(8):
                            nc.tensor.matmul(
                                ps_y[:],
                                on_sb[:, c, qb * 128 : (qb + 1) * 128],
                                wo_sb[:, c, fh * QT : (fh + 1) * QT],
                                start=(c == 0),
                                stop=(c == 7),
                            )
                        nc.vector.tensor_add(
                            y_sb[:, fh * QT : (fh + 1) * QT],
                            ps_y[:],
                            bo_sb[:, fh * QT : (fh + 1) * QT],
                        )
                    nc.sync.dma_start(y_out[qb * 128 : (qb + 1) * 128, :], y_sb[:])
            p3pool.__exit__(None, None, None)

    nc.compile()
    return nc


_PROGRAM_CACHE = {}


def _get_program():
    if "nc" not in _PROGRAM_CACHE:
        _PROGRAM_CACHE["nc"] = _build_program()
    return _PROGRAM_CACHE["nc"]


def _host_prep(x, idx, valid, geo_bias, Wq, Wk, Wv, Wo, bo):
    x2 = np.ascontiguousarray(np.asarray(x, dtype=np.float32).reshape(S, H))
    idx = np.asarray(idx).astype(np.int64)
    valid = np.asarray(valid).astype(bool)
    geo = np.asarray(geo_bias, dtype=np.float32)
    Wq = np.asarray(Wq, dtype=np.float32)
    Wk = np.asarray(Wk, dtype=np.float32)
    Wv = np.asarray(Wv, dtype=np.float32)
    Wo = np.asarray(Wo, dtype=np.float32)
    bo = np.asarray(bo, dtype=np.float32)

    qpos = np.arange(S, dtype=np.int64)[:, None]
    keep = valid & (idx <= qpos) & (idx >= 0)
    s_flat = idx[keep]
    q_flat = np.broadcast_to(qpos, idx.shape)[keep]
    lin = s_flat * S + q_flat

    bo_rep = np.ascontiguousarray(np.broadcast_to(bo[None, :], (128, H)))

    # den row order in phase 3 is r = l*8 + ci for head h = 2*ci + l
    sel16 = np.zeros((NH, H), dtype=np.float16)
    ch = np.arange(H)
    sel16[((ch // HD) % 2) * 8 + ch // 128, ch] = 1.0

    wq_scaled = Wq / np.sqrt(HD)
    # xT[p, c, s] = x[s, c*128+p]
    xT = np.ascontiguousarray(
        x2.T.reshape(8, 128, S).transpose(1, 0, 2)
    ).astype(np.float16)

    def wpack(W, cs):
        # w[p, c, m] = W[c*128+p, cols[m]]
        return np.ascontiguousarray(
            W[:, cs].reshape(8, 128, -1).transpose(1, 0, 2)
        ).astype(np.float16)

    wo_pack = np.ascontiguousarray(
        Wo.reshape(8, 128, H).transpose(1, 0, 2)
    ).astype(np.float16)

    in_maps = []
    for core in range(NC):
        e_pack = np.empty((N_TILES, SC, HPC, QT), dtype=np.float16)
        for l in range(HPC):
            h = HPC * core + l
            w = np.exp(geo[h][keep].astype(np.float64))
            eT = np.bincount(lin, weights=w, minlength=S * S).reshape(S, S)
            for n, (t, c) in enumerate(TILE_LIST):
                e_pack[n, :, l, :] = eT[
                    c * SC : (c + 1) * SC, t * QT : (t + 1) * QT
                ].astype(np.float16)
        cs = slice(128 * core, 128 * (core + 1))
        osel = np.zeros((128, 2), dtype=np.float32)
        osel[:, 0 if core < 4 else 1] = 1.0
        in_maps.append(
            {
                "xT": xT,
                "wq": wpack(wq_scaled, cs),
                "wk": wpack(Wk, cs),
                "wv": wpack(Wv, cs),
                "wo": wo_pack,
                "bo_rep": bo_rep,
                "sel16": sel16,
                "osel": osel,
                "e_pack": e_pack,
            }
        )
    return in_maps


def kernel(x, idx, valid, geo_bias, Wq, Wk, Wv, Wo, bo):
    b, s, h = np.asarray(x).shape
    assert (b, s, h) == (1, S, H)
    in_maps = _host_prep(x, idx, valid, geo_bias, Wq, Wk, Wv, Wo, bo)
    nc = _get_program()
    res = run_bass_kernel_spmd(nc, in_maps, core_ids=list(range(NC)))
    y = np.concatenate([res.results[c]["y_part"] for c in range(NC)], axis=0)
    return y.reshape(1, S, H).astype(np.float32)
